# revision 1
# baseline (speedup 1.0000x reference)
"""CKAM (DANet-style dual attention) Bass kernel for 8 trn2 NeuronCores.

Data-parallel over batch: each core processes one [512, 64, 64] image.

Per-core dataflow (N = H*W = 4096, C = 512, CH = 256, R = 64):
  Phase A: packed conv  [q|k](128, N)  = Wsp^T  @ [top;bottom]   (spatial q, k)
           chunk-outer accumulation, overlaps the input DMA stream.
  Phase B: transposed conv (N, 192) = [top;bottom]^T @ Wcsc      (qc^T, kc^T, ks^T)
  Phase C: conv          kc(64, N)  = Wkc^T @ [top;bottom]       (channel k)
  Channel attn:  scores = qc @ kc^T  (64x64), softmax, out_c = attn @ kc
  Spatial attn:  chunk pairs (even on PE rows 0:64, odd on rows 64:128 for
                 row-group concurrency): S = q^T k -> exp (ACT, accum d) ->
                 out_sp += (ks^T / d) contracted with E (col-group pairs)
  Final: out = [fs|fc] @ [out_sp; out_c] + bias   (single K=128 conv)

All 1x1 convs are folded through the (never materialized) x = top+bottom:
composite weights are computed on the host in float64.
"""

import numpy as np

import concourse.bass as bass
import concourse.bacc as bacc
import concourse.mybir as mybir
import concourse.tile as tile
from concourse import bass_utils
from concourse.bass import ts
from concourse.masks import make_identity

N_CORES = 8
C, HW = 512, 4096
CH, R = 256, 64
F32 = mybir.dt.float32
BF16 = mybir.dt.bfloat16
F32R = mybir.dt.float32r
EXP = mybir.ActivationFunctionType.Exp
AX = mybir.AxisListType.X

_CACHE: dict = {}

# Load top/bottom as bf16 (halves input DMA; rel err ~5e-3 vs ~2e-3)
INPUT_BF16 = True

ALL_PHASES = ("pa", "pb", "pc", "chan", "spatial", "final")


def build_program(phases=ALL_PHASES, repeat=1, input_bf16=None):
    if input_bf16 is None:
        input_bf16 = INPUT_BF16
    WDT = BF16 if input_bf16 else F32R
    WB = 192 if input_bf16 else 256
    IDT = BF16 if input_bf16 else F32R
    nc = bacc.Bacc("TRN2", target_bir_lowering=False, debug=False)

    top = nc.dram_tensor("top", (C, HW), IDT, kind="ExternalInput").ap()
    bot = nc.dram_tensor("bot", (C, HW), IDT, kind="ExternalInput").ap()
    wsp = nc.dram_tensor("wsp", (128, 8, 128), WDT, kind="ExternalInput").ap()
    wcsc = nc.dram_tensor("wcsc", (128, 8, WB), WDT, kind="ExternalInput").ap()
    wfa = nc.dram_tensor("wfa", (128, 4, 128), F32R, kind="ExternalInput").ap()
    wfb = nc.dram_tensor("wfb", (128, 4, 128), F32R, kind="ExternalInput").ap()
    b_qk = nc.dram_tensor("b_qk", (128, 1), F32, kind="ExternalInput").ap()
    b_csc = nc.dram_tensor("b_csc", (128, 192), F32, kind="ExternalInput").ap()
    b_kc = nc.dram_tensor("b_kc", (64, 1), F32, kind="ExternalInput").ap()
    b_f = nc.dram_tensor("b_f", (128, 4), F32, kind="ExternalInput").ap()
    out_d = nc.dram_tensor("out", (C, HW), F32, kind="ExternalOutput").ap()

    with tile.TileContext(nc) as tc:
      for _rep in range(repeat):
        with (
            tc.tile_pool(name="consts", bufs=1) as consts,
            tc.tile_pool(name="persist", bufs=1) as persist,
        ):
            wsp_sb = consts.tile([128, 8, 128], WDT)
            nc.sync.dma_start(out=wsp_sb, in_=wsp)
            wcsc_sb = consts.tile([128, 8, WB], WDT)
            nc.sync.dma_start(out=wcsc_sb, in_=wcsc)
            wfa_sb = consts.tile([128, 4, 128], F32R)
            nc.sync.dma_start(out=wfa_sb, in_=wfa)
            wfb_sb = consts.tile([128, 4, 128], F32R)
            nc.sync.dma_start(out=wfb_sb, in_=wfb)
            bqk_sb = consts.tile([128, 1], F32)
            nc.sync.dma_start(out=bqk_sb, in_=b_qk)
            bcsc_sb = consts.tile([128, 192], F32)
            nc.sync.dma_start(out=bcsc_sb, in_=b_csc)
            bkc_sb = consts.tile([64, 1], F32)
            nc.sync.dma_start(out=bkc_sb, in_=b_kc)
            bf_sb = consts.tile([128, 4], F32)
            nc.sync.dma_start(out=bf_sb, in_=b_f)
            ident = consts.tile([64, 64], F32)
            make_identity(nc, ident)

            # conv-phase outputs that live through the attention phase
            qk_sb = persist.tile([128, HW], BF16)  # q rows 0:64, k rows 64:128
            qk_swap = persist.tile([128, HW], BF16)  # [k | q] partition-swapped
            qckcT = persist.tile([128, 32, 128], F32)  # qc^T | kc^T  (n-major)
            ksT = persist.tile([128, 32, 64], BF16)  # spatial k^T
            kc_sb = persist.tile([64, HW], BF16)  # channel k
            stacked = persist.tile([128, HW], F32R)  # [out_sp|out_c] (swapped odd mb)

            # ---------------- conv phases (inputs resident) ----------------
            with tc.tile_pool(name="inputs", bufs=1) as inputs:
                top_r = top.rearrange("(a p) m -> a p m", p=128)
                bot_r = bot.rearrange("(a p) m -> a p m", p=128)
                srcs = [top_r[a] for a in range(4)] + [bot_r[a] for a in range(4)]
                if input_bf16:
                    chunks = []
                    for ci in range(8):
                        ch = inputs.tile([128, HW], BF16, tag=f"ch{ci}",
                                         name=f"ch{ci}")
                        nc.sync.dma_start(out=ch, in_=srcs[ci])
                        chunks.append(ch)
                else:
                    top_sb = inputs.tile([128, 4, HW], F32R)
                    bot_sb = inputs.tile([128, 4, HW], F32R)
                    for a in range(4):
                        nc.sync.dma_start(out=top_sb[:, a, :], in_=top_r[a])
                        nc.sync.dma_start(out=bot_sb[:, a, :], in_=bot_r[a])
                    chunks = [top_sb[:, a, :] for a in range(4)] + [
                        bot_sb[:, a, :] for a in range(4)
                    ]

                # Phase A (chunk-outer: starts as soon as chunk 0 lands)
                if "pa" in phases:
                    with tc.tile_pool(name="psA", bufs=1, space="PSUM") as psA:
                        psa_t = [
                            psA.tile([128, 512], F32, tag=f"a{mb}", name=f"psa{mb}")
                            for mb in range(8)
                        ]
                        for ci in range(8):
                            for mb in range(8):
                                nc.tensor.matmul(
                                    psa_t[mb],
                                    wsp_sb[:, ci, :],
                                    chunks[ci][:, ts(mb, 512)],
                                    start=(ci == 0),
                                    stop=(ci == 7),
                                )
                        for mb in range(8):
                            nc.vector.tensor_scalar_add(
                                qk_sb[:, ts(mb, 512)], psa_t[mb], bqk_sb
                            )
                    # [k|q] partition-swapped duplicate (SBUF->SBUF DMA)
                    nc.sync.dma_start(out=qk_swap[0:64, :], in_=qk_sb[64:128, :])
                    nc.sync.dma_start(out=qk_swap[64:128, :], in_=qk_sb[0:64, :])

                # Phase B: transposed convs -> qc^T | kc^T | ks^T
                if "pb" in phases:
                    with tc.tile_pool(name="psB", bufs=4, space="PSUM") as psB:
                        for nb in range(32):
                            ps = psB.tile([128, WB], F32, tag="b")
                            for ci in range(8):
                                nc.tensor.matmul(
                                    ps,
                                    chunks[ci][:, ts(nb, 128)],
                                    wcsc_sb[:, ci, :],
                                    start=(ci == 0),
                                    stop=(ci == 7),
                                )
                            nc.vector.tensor_add(
                                qckcT[:, nb, :], ps[:, 0:128], bcsc_sb[:, 0:128]
                            )
                            nc.vector.tensor_add(
                                ksT[:, nb, :], ps[:, 128:192], bcsc_sb[:, 128:192]
                            )

                # Phase C: channel kc conv -> [64, HW]
                if "pc" in phases:
                    with tc.tile_pool(name="psC", bufs=4, space="PSUM") as psC:
                        for mb in range(8):
                            ps = psC.tile([64, 512], F32, tag="c")
                            for ci in range(8):
                                nc.tensor.matmul(
                                    ps,
                                    wcsc_sb[:, ci, 64:128],
                                    chunks[ci][:, ts(mb, 512)],
                                    start=(ci == 0),
                                    stop=(ci == 7),
                                )
                            nc.vector.tensor_scalar_add(
                                kc_sb[:, ts(mb, 512)], ps, bkc_sb
                            )

            # ---------------- channel attention ----------------
            if "chan" not in phases:
                nc.vector.memset(stacked.bitcast(F32), 0.0)
            if "chan" in phases:
                with (
                    tc.tile_pool(name="chan", bufs=1) as chs,
                    tc.tile_pool(name="chp", bufs=1, space="PSUM") as chp,
                    tc.tile_pool(name="chop", bufs=2, space="PSUM") as chop,
                ):
                    sc_ps = chp.tile([64, 64], F32, tag="sc")
                    for nb in range(32):
                        nc.tensor.matmul(
                            sc_ps,
                            qckcT[:, nb, 0:64],
                            qckcT[:, nb, 64:128],
                            start=(nb == 0),
                            stop=(nb == 31),
                        )
                    sc = chs.tile([64, 64], F32)
                    nc.vector.tensor_copy(sc, sc_ps)
                    mx = chs.tile([64, 1], F32)
                    nc.vector.reduce_max(mx, sc, axis=AX)
                    negmx = chs.tile([64, 1], F32)
                    nc.vector.tensor_scalar_mul(negmx, mx, -1.0)
                    ec = chs.tile([64, 64], F32)
                    dc = chs.tile([64, 1], F32)
                    nc.scalar.activation(
                        ec, sc, EXP, bias=negmx, scale=1.0, accum_out=dc
                    )
                    rdc = chs.tile([64, 1], F32)
                    nc.vector.reciprocal(rdc, dc)
                    ac = chs.tile([64, 64], F32)
                    nc.vector.tensor_scalar_mul(ac, ec, rdc)
                    acT_ps = chp.tile([64, 64], F32, tag="acT")
                    nc.tensor.transpose(acT_ps, ac, ident)
                    acT = chs.tile([64, 64], BF16)
                    nc.vector.tensor_copy(acT, acT_ps)
                    # out_c: even mb -> partitions 64:128, odd mb -> 0:64
                    for j in range(4):
                        ps = chop.tile([128, 512], F32, tag="oc")
                        nc.tensor.matmul(
                            ps[64:128, :], acT, kc_sb[:, ts(2 * j, 512)],
                            start=True, stop=True, skip_group_check=True,
                        )
                        nc.tensor.matmul(
                            ps[0:64, :], acT, kc_sb[:, ts(2 * j + 1, 512)],
                            start=True, stop=True, skip_group_check=True,
                        )
                        nc.vector.tensor_copy(
                            stacked[64:128, ts(2 * j, 512)], ps[64:128, :]
                        )
                        nc.vector.tensor_copy(
                            stacked[0:64, ts(2 * j + 1, 512)], ps[0:64, :]
                        )

            # ---------------- spatial attention ----------------
            if "spatial" in phases:
                with (
                    tc.tile_pool(name="spE", bufs=2) as spp,
                    tc.tile_pool(name="spS", bufs=2) as sps,
                    tc.tile_pool(name="psSe", bufs=1, space="PSUM") as psSe,
                    tc.tile_pool(name="psSo", bufs=1, space="PSUM") as psSo,
                    tc.tile_pool(name="psO", bufs=1, space="PSUM") as psO,
                ):
                    out_ps = [
                        psO.tile([128, 512], F32, tag=f"o{j}", name=f"out_ps{j}")
                        for j in range(4)
                    ]
                    # chunk pairs: even chunk on PE rows 0:64, odd on 64:128
                    for t in range(16):
                        i_e, i_o = 2 * t, 2 * t + 1
                        E_e = spp.tile([128, HW], BF16, tag="Ee", name="E_e")
                        E_o = spp.tile([128, HW], BF16, tag="Eo", name="E_o")
                        dp_e = sps.tile([128, 4], F32, tag="dpe", name="dp_e")
                        dp_o = sps.tile([128, 4], F32, tag="dpo", name="dp_o")
                        for q in range(4):
                            s_e = psSe.tile([128, 1024], F32, tag="se", name="s_e")
                            s_o = psSo.tile([128, 1024], F32, tag="so", name="s_o")
                            for jm in range(2):
                                mb = 2 * q + jm
                                nc.tensor.matmul(
                                    s_e[:, ts(jm, 512)],
                                    qk_sb[0:64, ts(i_e, 128)],
                                    qk_swap[0:64, ts(mb, 512)],
                                    start=True,
                                    stop=True,
                                )
                                nc.tensor.matmul(
                                    s_o[:, ts(jm, 512)],
                                    qk_swap[64:128, ts(i_o, 128)],
                                    qk_sb[64:128, ts(mb, 512)],
                                    start=True,
                                    stop=True,
                                )
                            nc.scalar.activation(
                                E_e[:, ts(q, 1024)], s_e, EXP,
                                accum_out=dp_e[:, q : q + 1],
                            )
                            nc.scalar.activation(
                                E_o[:, ts(q, 1024)], s_o, EXP,
                                accum_out=dp_o[:, q : q + 1],
                            )
                        for par, i_c, dp, E in (
                            ("e", i_e, dp_e, E_e),
                            ("o", i_o, dp_o, E_o),
                        ):
                            d = sps.tile([128, 1], F32, tag=f"d{par}", name="d")
                            nc.vector.reduce_sum(d, dp, axis=AX)
                            rd = sps.tile([128, 1], F32, tag=f"rd{par}", name="rd")
                            nc.vector.reciprocal(rd, d)
                            kst = sps.tile(
                                [128, 64], BF16, tag=f"kst{par}", name="kst"
                            )
                            nc.vector.tensor_scalar_mul(kst, ksT[:, i_c, :], rd)
                            for j in range(4):
                                nc.tensor.matmul(
                                    out_ps[j][0:64, :], kst, E[:, ts(2 * j, 512)],
                                    start=(i_c == 0), stop=(i_c == 31),
                                    skip_group_check=True,
                                )
                                nc.tensor.matmul(
                                    out_ps[j][64:128, :],
                                    kst,
                                    E[:, ts(2 * j + 1, 512)],
                                    start=(i_c == 0), stop=(i_c == 31),
                                    skip_group_check=True,
                                )
                    for j in range(4):
                        nc.vector.tensor_copy(
                            stacked[0:64, ts(2 * j, 512)], out_ps[j][0:64, :]
                        )
                        nc.vector.tensor_copy(
                            stacked[64:128, ts(2 * j + 1, 512)],
                            out_ps[j][64:128, :],
                        )

            # ---------------- final fused conv ----------------
            if "final" in phases:
                with (
                    tc.tile_pool(name="fin", bufs=4) as fins,
                    tc.tile_pool(name="psF", bufs=4, space="PSUM") as psF,
                ):
                    out_r = out_d.rearrange("(k p) m -> k p m", p=128)
                    for mb in range(8):
                        wf = wfa_sb if mb % 2 == 0 else wfb_sb
                        for cok in range(4):
                            ps = psF.tile([128, 512], F32, tag="f")
                            nc.tensor.matmul(
                                ps,
                                wf[:, cok, :],
                                stacked[:, ts(mb, 512)],
                                start=True, stop=True,
                            )
                            ft = fins.tile([128, 512], F32, tag="ft")
                            if (mb * 4 + cok) % 2 == 0:
                                nc.vector.tensor_scalar_add(
                                    ft, ps, bf_sb[:, cok : cok + 1]
                                )
                            else:
                                nc.scalar.add(ft, ps, bf_sb[:, cok : cok + 1])
                            nc.sync.dma_start(
                                out=out_r[cok, :, ts(mb, 512)], in_=ft
                            )

    nc.compile()
    return nc


def make_weight_arrays(inputs):
    """Host-side composite weights (float64 accumulate, float32 out)."""
    f8 = lambda a: np.asarray(a, dtype=np.float64)
    wt, wb = f8(inputs["wt"]), f8(inputs["wb"])
    bt, bb = f8(inputs["bt"]), f8(inputs["bb"])
    s_w1, s_b1 = f8(inputs["s_w1"]), f8(inputs["s_b1"])
    s_w2, s_b2 = f8(inputs["s_w2"]), f8(inputs["s_b2"])
    s_wo, s_bo = f8(inputs["s_wo"]), f8(inputs["s_bo"])
    c_wq, c_bq = f8(inputs["c_wq"]), f8(inputs["c_bq"])
    c_wk, c_bk = f8(inputs["c_wk"]), f8(inputs["c_bk"])
    c_wo, c_bo = f8(inputs["c_wo"]), f8(inputs["c_bo"])
    f_w, f_b = f8(inputs["f_w"]), f8(inputs["f_b"])

    wt1, wt2 = wt[:CH], wt[CH:]
    wb1, wb2 = wb[:CH], wb[CH:]
    btb = bt + bb
    btb1, btb2 = btb[:CH], btb[CH:]

    A_q, B_q = s_w1 @ wt1, s_w1 @ wb1
    A_k, B_k = s_w2 @ wt1, s_w2 @ wb1
    C_q, D_q = c_wq @ wt2, c_wq @ wb2
    C_k, D_k = c_wk @ wt2, c_wk @ wb2

    wsp_full = np.concatenate(
        [
            np.concatenate([A_q.T, A_k.T], axis=1),
            np.concatenate([B_q.T, B_k.T], axis=1),
        ],
        axis=0,
    )  # [1024, 128]
    wsp = wsp_full.reshape(8, 128, 128).transpose(1, 0, 2)

    bias_q = s_w1 @ btb1 + s_b1
    bias_k = s_w2 @ btb1 + s_b2
    b_qk = np.concatenate([bias_q, bias_k])[:, None]

    wcsc_full = np.concatenate(
        [
            np.concatenate([C_q.T, C_k.T, A_k.T], axis=1),
            np.concatenate([D_q.T, D_k.T, B_k.T], axis=1),
        ],
        axis=0,
    )  # [1024, 192]
    wb_width = 192 if INPUT_BF16 else 256
    wcsc = np.zeros((8, 128, wb_width), np.float64)
    wcsc[:, :, :192] = wcsc_full.reshape(8, 128, 192)
    wcsc = wcsc.transpose(1, 0, 2)

    bias_qc = c_wq @ btb2 + c_bq
    bias_kc = c_wk @ btb2 + c_bk
    bcsc_vec = np.concatenate([bias_qc, bias_kc, bias_k])  # [192]
    b_csc = np.broadcast_to(bcsc_vec, (128, 192)).copy()
    b_kc = bias_kc[:, None]

    fs = f_w[:, :CH] @ s_wo  # [512, 64]
    fc = f_w[:, CH:] @ c_wo
    wfa = np.concatenate([fs, fc], axis=1).T.reshape(128, 4, 128)
    wfb = np.concatenate([fc, fs], axis=1).T.reshape(128, 4, 128)
    bias_f = f_w[:, :CH] @ s_bo + f_w[:, CH:] @ c_bo + f_b  # [512]
    b_f = bias_f.reshape(4, 128).T

    import ml_dtypes

    cast = lambda a: np.ascontiguousarray(a, dtype=np.float32)
    wdt = ml_dtypes.bfloat16 if INPUT_BF16 else np.float32
    wcast = lambda a: np.ascontiguousarray(a.astype(np.float32), dtype=wdt)
    return {
        "wsp": wcast(wsp),
        "wcsc": wcast(wcsc),
        "wfa": cast(wfa),
        "wfb": cast(wfb),
        "b_qk": cast(b_qk),
        "b_csc": cast(b_csc),
        "b_kc": cast(b_kc),
        "b_f": cast(b_f),
    }


def kernel(**inputs):
    if "nc" not in _CACHE:
        _CACHE["nc"] = build_program()
    nc = _CACHE["nc"]

    import ml_dtypes

    weights = make_weight_arrays(inputs)
    idt = ml_dtypes.bfloat16 if INPUT_BF16 else np.float32
    top_all = np.ascontiguousarray(
        np.asarray(inputs["top_feat"], dtype=np.float32)
        .reshape(N_CORES, C, HW)
        .astype(idt)
    )
    bot_all = np.ascontiguousarray(
        np.asarray(inputs["bottom_feat"], dtype=np.float32)
        .reshape(N_CORES, C, HW)
        .astype(idt)
    )
    in_maps = [
        {"top": top_all[b], "bot": bot_all[b], **weights} for b in range(N_CORES)
    ]
    res = bass_utils.run_bass_kernel_spmd(nc, in_maps, core_ids=list(range(N_CORES)))
    out = np.stack([res.results[b]["out"] for b in range(N_CORES)])
    return out.reshape(N_CORES, C, 64, 64)



# revision 35
# speedup vs baseline: 1.6474x; 1.6474x over previous
"""CKAM (DANet-style dual attention) Bass kernel for 8 trn2 NeuronCores.

Data-parallel over batch: each core processes one [512, 64, 64] image.

Per-core dataflow (N = H*W = 4096, C = 512, CH = 256, R = 64). All 1x1
convs are folded through the (never materialized) x = top+bottom into
composite weights computed on the host in float64. Note v = k in both
attention branches of the reference, so only four conv outputs exist:
q, k (spatial) and qc, kc (channel), i.e. TWO 128-wide convs.

  conv1: [q|k]  (128, N) = Wqk^T  @ [top;bottom]   (chunk-outer accumulation
         overlapping the input DMA stream; ci==7 tail pipelined per m-block)
  conv2: [qc|kc](128, N) = Wqc^T  @ [top;bottom]   (eighths, interleaved into
         spatial chunks 0..7, using the psO banks before out-accum starts)
  kT / qckcT: DMA xbar transposes of the conv outputs; transposed row n
         lands at (partition n%128, chunk n//128), i.e. chunk c of ksT/qckcT
         holds pixel columns 128c..128c+127.
  Spatial attn, single chunk stream, double-buffered S PSUM (2x 2 banks)
         keeps ACT (exp) saturated: S = q^T k -> exp (ACT, accum d) ->
         out_sp += (kT/d)^T @ E. The out-matmuls of chunk c are emitted
         after the S-matmuls of chunk c+1 (software pipelining) so the
         in-order PE queue never blocks on the current chunk's exp; the
         backlog for chunks 0..7 drains 2/iter over chunks 8..15.
  Channel attn: scores = qc @ kc^T (64x64), softmax, out_c = attn @ kc;
         runs in the spatial tail on freed S PSUM slots.
  Final: out = [fs|fc] @ [out_sp; out_c] + bias (K=128 conv), staged bf16
         out DMA ([128,2048] groups), host casts to f32.
"""

import numpy as np

import concourse.bass as bass
import concourse.bacc as bacc
import concourse.mybir as mybir
import concourse.tile as tile
from concourse import bass_utils
from concourse.bass import ts
from concourse.masks import make_identity

N_CORES = 8
C, HW = 512, 4096
CH, R = 256, 64
F32 = mybir.dt.float32
BF16 = mybir.dt.bfloat16
EXP = mybir.ActivationFunctionType.Exp
AX = mybir.AxisListType.X

_CACHE: dict = {}

# how the softmax denominator is produced:
#   "accum"  - ACT accum_out on each exp (costs ~187ns/activation on ACT)
#   "reduce" - DVE reduce_sum over the bf16 E tile in SBUF
D_MODE = "accum"


def build_program(repeat=1, d_mode=None):
    if d_mode is None:
        d_mode = D_MODE
    IDT = BF16
    WDT = BF16
    nc = bacc.Bacc("TRN2", target_bir_lowering=False, debug=False)

    top = nc.dram_tensor("top", (C, HW), IDT, kind="ExternalInput").ap()
    bot = nc.dram_tensor("bot", (C, HW), IDT, kind="ExternalInput").ap()
    wqk = nc.dram_tensor("wqk", (128, 8, 128), WDT, kind="ExternalInput").ap()
    wqc = nc.dram_tensor("wqc", (128, 8, 128), WDT, kind="ExternalInput").ap()
    wfa = nc.dram_tensor("wfa", (128, 4, 128), BF16, kind="ExternalInput").ap()
    wfb = nc.dram_tensor("wfb", (128, 4, 128), BF16, kind="ExternalInput").ap()
    b_qk = nc.dram_tensor("b_qk", (128, 1), F32, kind="ExternalInput").ap()
    b_qc = nc.dram_tensor("b_qc", (128, 1), F32, kind="ExternalInput").ap()
    b_f = nc.dram_tensor("b_f", (128, 4), F32, kind="ExternalInput").ap()
    out_d = nc.dram_tensor("out", (C, HW), BF16, kind="ExternalOutput").ap()

    with tile.TileContext(nc) as tc:
      for _rep in range(repeat):
        with (
            tc.tile_pool(name="consts", bufs=1) as consts,
            tc.tile_pool(name="persist", bufs=1) as persist,
        ):
            wqk_sb = consts.tile([128, 8, 128], WDT)
            nc.sync.dma_start(out=wqk_sb, in_=wqk)
            wqc_sb = consts.tile([128, 8, 128], WDT)
            nc.sync.dma_start(out=wqc_sb, in_=wqc)
            wfa_sb = consts.tile([128, 4, 128], BF16)
            nc.sync.dma_start(out=wfa_sb, in_=wfa)
            wfb_sb = consts.tile([128, 4, 128], BF16)
            nc.sync.dma_start(out=wfb_sb, in_=wfb)
            bqk_sb = consts.tile([128, 1], F32)
            nc.sync.dma_start(out=bqk_sb, in_=b_qk)
            bqc_sb = consts.tile([128, 1], F32)
            nc.sync.dma_start(out=bqc_sb, in_=b_qc)
            bf_sb = consts.tile([128, 4], F32)
            nc.sync.dma_start(out=bf_sb, in_=b_f)
            ident = consts.tile([64, 64], F32)
            make_identity(nc, ident)
            # preload the exp table set during the DMA head
            warm = consts.tile([128, 1], F32)
            nc.scalar.activation(warm, bqk_sb, EXP)

            qk_sb = persist.tile([128, HW], BF16)   # q rows 0:64, k rows 64:128
            qk_swap = persist.tile([128, HW], BF16)  # [k | q] partition-swapped
            qckc_sb = persist.tile([128, HW], BF16)  # qc rows 0:64, kc 64:128
            qckcT = persist.tile([128, 32, 128], BF16)  # qc^T|kc^T (n=32p+c)
            ksT = persist.tile([128, 32, 64], BF16)     # k^T      (n=32p+c)
            stacked = persist.tile([128, HW], BF16)  # [out_sp|out_c] (swap odd)
            kc_lo = persist.tile([128, HW], BF16)  # kc on partitions 0:64

            with tc.tile_pool(name="inputs", bufs=1) as inputs:
                top_r = top.rearrange("(a p) m -> a p m", p=128)
                bot_r = bot.rearrange("(a p) m -> a p m", p=128)
                srcs = [top_r[a] for a in range(4)] + [bot_r[a] for a in range(4)]
                chunks = []
                for ci in range(8):
                    ch = inputs.tile([128, HW], IDT, tag=f"ch{ci}", name=f"ch{ci}")
                    # split each chunk DMA so conv1 matmuls can start on a
                    # block as soon as its columns land (deps are AP-ranged);
                    # the last chunk gates the head, so split it finest
                    npiece = 4 if ci == 7 else 2
                    w = HW // npiece
                    for g in range(npiece):
                        nc.sync.dma_start(
                            out=ch[:, ts(g, w)], in_=srcs[ci][:, ts(g, w)]
                        )
                    chunks.append(ch)

                # ---------- conv1: [q|k] (chunk-outer accumulation) ----------
                # the ci==7 tail is pipelined per m-block: matmul -> bias add
                # (alternating DVE/ACT) -> per-block qk_swap DMAs + ksT
                # transpose, so the first S matmuls start as early as possible
                with tc.tile_pool(name="psA", bufs=1, space="PSUM") as psA:
                    psa_t = [
                        psA.tile([128, 512], F32, tag=f"a{mb}", name=f"psa{mb}")
                        for mb in range(8)
                    ]
                    for ci in range(7):
                        for mb in range(8):
                            nc.tensor.matmul(
                                psa_t[mb],
                                wqk_sb[:, ci, :],
                                chunks[ci][:, ts(mb, 512)],
                                start=(ci == 0),
                                stop=False,
                            )
                    for mb in range(8):
                        nc.tensor.matmul(
                            psa_t[mb],
                            wqk_sb[:, 7, :],
                            chunks[7][:, ts(mb, 512)],
                            start=False,
                            stop=True,
                        )
                        if mb % 2 == 0:
                            nc.vector.tensor_scalar_add(
                                qk_sb[:, ts(mb, 512)], psa_t[mb], bqk_sb
                            )
                        else:
                            nc.scalar.add(
                                qk_sb[:, ts(mb, 512)], psa_t[mb], bqk_sb
                            )
                        # k copied down to partitions 0:64 (SBUF->SBUF DMA) so
                        # S matmuls can pair it with q (also on 0:64)
                        nc.sync.dma_start(
                            out=qk_swap[0:64, ts(mb, 512)],
                            in_=qk_sb[64:128, ts(mb, 512)],
                        )
                    # k^T via a single DMA xbar transpose (chunk c of ksT
                    # holds pixel columns 128c..: ksT[p,c,:] = k[:,128c+p]).
                    # One transpose, not one per block: every transition
                    # between xbar and copy mode drains the DMA queue.
                    nc.sync.dma_start(
                        out=ksT, in_=qk_sb[64:128, :], transpose=True
                    )

                # ---------------- spatial attention ----------------
                # chunk c = pixel columns 128c..128c+127 (matches the DMA
                # transpose layout: ksT[p, c, :] = k[:, 128c + p]). Single
                # chunk stream, double-buffered S PSUM (2x 2 banks) keeps the
                # scalar engine (exp) saturated; conv2 eighths ride along in
                # chunks 0..7.
                with (
                    tc.tile_pool(name="spE", bufs=10) as spp,
                    tc.tile_pool(name="spS", bufs=2) as sps,
                    tc.tile_pool(name="psS", bufs=2, space="PSUM") as psS,
                    tc.tile_pool(name="psO", bufs=1, space="PSUM") as psO,
                ):
                  # conv2 eighths use the psO banks while they are still
                  # free; the out-matmul backlog for chunks 0..7 drains two
                  # per iteration over chunks 8..15 (E tiles buffered deep)
                  out_ps = [None] * 4
                  Es, ksts = {}, {}

                  def emit_out_mms(cp):
                      Ep, kstp = Es.pop(cp), ksts.pop(cp)
                      for j in range(4):
                          nc.tensor.matmul(
                              out_ps[j][0:64, :], kstp,
                              Ep[:, ts(2 * j, 512)],
                              start=(cp == 0), stop=(cp == 31),
                              skip_group_check=True,
                          )
                          nc.tensor.matmul(
                              out_ps[j][64:128, :],
                              kstp,
                              Ep[:, ts(2 * j + 1, 512)],
                              start=(cp == 0), stop=(cp == 31),
                              skip_group_check=True,
                          )

                  for c in range(34):
                      if c == 9:
                          for j in range(4):
                              out_ps[j] = psO.tile(
                                  [128, 512], F32, tag=f"o{j}",
                                  name=f"out_ps{j}",
                              )
                      if c < 32:
                          E = spp.tile([128, HW], BF16, tag="E", name="E")
                          Es[c] = E
                          # d = rowsum(exp(S)). Chunks 0..7: ACT accumulator
                          # (DVE busy with conv2 bias adds). Chunks 8..31:
                          # plain exps + two pipelined half-row DVE reduces
                          # over the bf16 E tile -- saves the 187ns/activation
                          # accumulator read on the ACT critical path.
                          use_reduce = True
                          dp = sps.tile([128, 4], F32, tag="dp", name="dp")
                          for q in range(4):
                              s = psS.tile([128, 1024], F32, tag="s", name="s")
                              for jm in range(2):
                                  mb = 2 * q + jm
                                  nc.tensor.matmul(
                                      s[:, ts(jm, 512)],
                                      qk_sb[0:64, ts(c, 128)],
                                      qk_swap[0:64, ts(mb, 512)],
                                      start=True,
                                      stop=True,
                                  )
                              if use_reduce and q < 3:
                                  # q0..q2: plain exp; DVE row-sums trail the
                                  # E stream (cols of q0+q1, then q2)
                                  nc.scalar.activation(
                                      E[:, ts(q, 1024)], s, EXP
                                  )
                                  if q == 1:
                                      dh0 = sps.tile(
                                          [128, 1], F32, tag="dh0", name="dh0"
                                      )
                                      nc.vector.reduce_sum(
                                          dh0, E[:, 0:2048], axis=AX
                                      )
                                  elif q == 2:
                                      dh1 = sps.tile(
                                          [128, 1], F32, tag="dh1", name="dh1"
                                      )
                                      nc.vector.reduce_sum(
                                          dh1, E[:, 2048:3072], axis=AX
                                      )
                              else:
                                  # q3 (and all of chunks 0..7) keeps the ACT
                                  # accumulator so d completes ~immediately
                                  # after the last exp of the chunk
                                  nc.scalar.activation(
                                      E[:, ts(q, 1024)], s, EXP,
                                      accum_out=dp[:, q : q + 1],
                                  )
                          d = sps.tile([128, 1], F32, tag="d", name="d")
                          if use_reduce:
                              dtmp = sps.tile(
                                  [128, 1], F32, tag="dtmp", name="dtmp"
                              )
                              nc.vector.tensor_add(dtmp, dh0, dh1)
                              nc.vector.tensor_add(d, dtmp, dp[:, 3:4])
                          else:
                              nc.vector.reduce_sum(d, dp, axis=AX)
                          rd = sps.tile([128, 1], F32, tag="rd", name="rd")
                          nc.vector.reciprocal(rd, d)
                          kst = sps.tile([128, 64], BF16, tag="kst", name="kst", bufs=3)
                          nc.vector.tensor_scalar_mul(kst, ksT[:, c, :], rd)
                          ksts[c] = kst
                          # conv2 eighth (one m-block) in a free psO bank
                          if c < 8:
                              ps2 = psO.tile(
                                  [128, 512], F32, tag=f"o{c % 4}", name="ps2"
                              )
                              for ci in range(8):
                                  nc.tensor.matmul(
                                      ps2,
                                      wqc_sb[:, ci, :],
                                      chunks[ci][:, ts(c, 512)],
                                      start=(ci == 0),
                                      stop=(ci == 7),
                                  )
                              nc.vector.tensor_scalar_add(
                                  qckc_sb[:, ts(c, 512)], ps2, bqc_sb
                              )
                              if c == 7:
                                  # qc^T|kc^T via DMA xbar transpose
                                  nc.sync.dma_start(
                                      out=qckcT, in_=qckc_sb, transpose=True
                                  )
                                  # kc down to partitions 0:64 for the
                                  # channel value matmuls (PE transpose out
                                  # must sit at PSUM partition 0)
                                  nc.sync.dma_start(
                                      out=kc_lo[0:64, :],
                                      in_=qckc_sb[64:128, :],
                                  )
                      if 9 <= c <= 16:
                          emit_out_mms(2 * (c - 9))
                          emit_out_mms(2 * (c - 9) + 1)
                      elif c >= 18:
                          emit_out_mms(c - 2)
                  # drain out_sp accumulators to SBUF (alternate DVE / ACT)
                  for j in range(4):
                      nc.vector.tensor_copy(
                          stacked[0:64, ts(2 * j, 512)], out_ps[j][0:64, :]
                      )
                      nc.scalar.copy(
                          stacked[64:128, ts(2 * j + 1, 512)],
                          out_ps[j][64:128, :],
                      )

                  # ---------------- channel attention ----------------
                  # runs in the spatial tail, borrowing the freed S PSUM
                  # slots; overlaps the out(31) matmuls and drain copies
                  sc_ps = psS.tile([128, 1024], F32, tag="s", name="sc_ps")
                  for nb in range(32):
                      nc.tensor.matmul(
                          sc_ps[0:64, 0:64],
                          qckcT[:, nb, 0:64],
                          qckcT[:, nb, 64:128],
                          start=(nb == 0),
                          stop=(nb == 31),
                      )
                  sc = sps.tile([64, 64], F32, tag="sc", name="sc")
                  nc.vector.tensor_copy(sc, sc_ps[0:64, 0:64])
                  mx = sps.tile([64, 1], F32, tag="mx", name="mx")
                  nc.vector.reduce_max(mx, sc, axis=AX)
                  negmx = sps.tile([64, 1], F32, tag="negmx", name="negmx")
                  nc.vector.tensor_scalar_mul(negmx, mx, -1.0)
                  ec = sps.tile([64, 64], F32, tag="ec", name="ec")
                  dc = sps.tile([64, 1], F32, tag="dc", name="dc")
                  nc.scalar.activation(
                      ec, sc, EXP, bias=negmx, scale=1.0, accum_out=dc
                  )
                  rdc = sps.tile([64, 1], F32, tag="rdc", name="rdc")
                  nc.vector.reciprocal(rdc, dc)
                  ac = sps.tile([64, 64], F32, tag="ac", name="ac")
                  nc.vector.tensor_scalar_mul(ac, ec, rdc)
                  acT_ps = psS.tile([128, 1024], F32, tag="s", name="acT_ps")
                  nc.tensor.transpose(acT_ps[0:64, 0:64], ac, ident)
                  acT = sps.tile([64, 64], BF16, tag="acT", name="acT")
                  nc.vector.tensor_copy(acT, acT_ps[0:64, 0:64])
                  # out_c: even mb -> partitions 64:128, odd mb -> 0:64
                  kc = kc_lo[0:64, :]
                  for jj in range(2):
                      oc = psS.tile([128, 1024], F32, tag="s", name="oc")
                      for jh in range(2):
                          j = 2 * jj + jh
                          nc.tensor.matmul(
                              oc[64:128, ts(jh, 512)], acT,
                              kc[:, ts(2 * j, 512)],
                              start=True, stop=True, skip_group_check=True,
                          )
                          nc.tensor.matmul(
                              oc[0:64, ts(jh, 512)], acT,
                              kc[:, ts(2 * j + 1, 512)],
                              start=True, stop=True, skip_group_check=True,
                          )
                          nc.vector.tensor_copy(
                              stacked[64:128, ts(2 * j, 512)],
                              oc[64:128, ts(jh, 512)],
                          )
                          nc.scalar.copy(
                              stacked[0:64, ts(2 * j + 1, 512)],
                              oc[0:64, ts(jh, 512)],
                          )

            # ---------------- final fused conv ----------------
            with (
                tc.tile_pool(name="fin", bufs=4) as fins,
                tc.tile_pool(name="psF", bufs=4, space="PSUM") as psF,
            ):
                out_r = out_d.rearrange("(k p) m -> k p m", p=128)
                # cok-outer; stage 4 m-blocks per output DMA so the tail pays
                # 8 HWDGE dispatches instead of 32
                for cok in range(4):
                    for half in range(2):
                        ft = fins.tile([128, 2048], BF16, tag="ft", name="ft")
                        for jp in range(2):
                            ps = psF.tile([128, 1024], F32, tag="f")
                            for jm in (2 * jp, 2 * jp + 1):
                                mb = 4 * half + jm
                                wf = wfa_sb if mb % 2 == 0 else wfb_sb
                                nc.tensor.matmul(
                                    ps[:, ts(jm - 2 * jp, 512)],
                                    wf[:, cok, :],
                                    stacked[:, ts(mb, 512)],
                                    start=True, stop=True,
                                    skip_group_check=True,
                                )
                            # one [128,1024] bias add per psF tile (bias only
                            # depends on cok, so it spans both m-blocks)
                            if (2 * half + jp + cok) % 2 == 0:
                                nc.vector.tensor_scalar_add(
                                    ft[:, ts(jp, 1024)], ps,
                                    bf_sb[:, cok : cok + 1],
                                )
                            else:
                                nc.scalar.add(
                                    ft[:, ts(jp, 1024)], ps,
                                    bf_sb[:, cok : cok + 1],
                                )
                        nc.sync.dma_start(
                            out=out_r[cok, :, ts(half, 2048)], in_=ft
                        )

    nc.compile()
    return nc


def make_weight_arrays(inputs):
    """Host-side composite weights (float64 accumulate, bf16/f32 out)."""
    f8 = lambda a: np.asarray(a, dtype=np.float64)
    wt, wb = f8(inputs["wt"]), f8(inputs["wb"])
    bt, bb = f8(inputs["bt"]), f8(inputs["bb"])
    s_w1, s_b1 = f8(inputs["s_w1"]), f8(inputs["s_b1"])
    s_w2, s_b2 = f8(inputs["s_w2"]), f8(inputs["s_b2"])
    s_wo, s_bo = f8(inputs["s_wo"]), f8(inputs["s_bo"])
    c_wq, c_bq = f8(inputs["c_wq"]), f8(inputs["c_bq"])
    c_wk, c_bk = f8(inputs["c_wk"]), f8(inputs["c_bk"])
    c_wo, c_bo = f8(inputs["c_wo"]), f8(inputs["c_bo"])
    f_w, f_b = f8(inputs["f_w"]), f8(inputs["f_b"])

    wt1, wt2 = wt[:CH], wt[CH:]
    wb1, wb2 = wb[:CH], wb[CH:]
    btb = bt + bb
    btb1, btb2 = btb[:CH], btb[CH:]

    A_q, B_q = s_w1 @ wt1, s_w1 @ wb1
    A_k, B_k = s_w2 @ wt1, s_w2 @ wb1
    C_q, D_q = c_wq @ wt2, c_wq @ wb2
    C_k, D_k = c_wk @ wt2, c_wk @ wb2

    wqk_full = np.concatenate(
        [
            np.concatenate([A_q.T, A_k.T], axis=1),
            np.concatenate([B_q.T, B_k.T], axis=1),
        ],
        axis=0,
    )  # [1024, 128]
    wqk = wqk_full.reshape(8, 128, 128).transpose(1, 0, 2)

    wqc_full = np.concatenate(
        [
            np.concatenate([C_q.T, C_k.T], axis=1),
            np.concatenate([D_q.T, D_k.T], axis=1),
        ],
        axis=0,
    )  # [1024, 128]
    wqc = wqc_full.reshape(8, 128, 128).transpose(1, 0, 2)

    bias_q = s_w1 @ btb1 + s_b1
    bias_k = s_w2 @ btb1 + s_b2
    b_qk = np.concatenate([bias_q, bias_k])[:, None]
    bias_qc = c_wq @ btb2 + c_bq
    bias_kc = c_wk @ btb2 + c_bk
    b_qc = np.concatenate([bias_qc, bias_kc])[:, None]

    fs = f_w[:, :CH] @ s_wo  # [512, 64]
    fc = f_w[:, CH:] @ c_wo
    wfa = np.concatenate([fs, fc], axis=1).T.reshape(128, 4, 128)
    wfb = np.concatenate([fc, fs], axis=1).T.reshape(128, 4, 128)
    bias_f = f_w[:, :CH] @ s_bo + f_w[:, CH:] @ c_bo + f_b  # [512]
    b_f = bias_f.reshape(4, 128).T

    import ml_dtypes

    cast = lambda a: np.ascontiguousarray(a, dtype=np.float32)
    wcast = lambda a: np.ascontiguousarray(
        a.astype(np.float32), dtype=ml_dtypes.bfloat16
    )
    return {
        "wqk": wcast(wqk),
        "wqc": wcast(wqc),
        "wfa": wcast(wfa),
        "wfb": wcast(wfb),
        "b_qk": cast(b_qk),
        "b_qc": cast(b_qc),
        "b_f": cast(b_f),
    }


def kernel(**inputs):
    if "nc" not in _CACHE:
        _CACHE["nc"] = build_program()
    nc = _CACHE["nc"]

    import ml_dtypes

    weights = make_weight_arrays(inputs)
    top_all = np.ascontiguousarray(
        np.asarray(inputs["top_feat"], dtype=np.float32)
        .reshape(N_CORES, C, HW)
        .astype(ml_dtypes.bfloat16)
    )
    bot_all = np.ascontiguousarray(
        np.asarray(inputs["bottom_feat"], dtype=np.float32)
        .reshape(N_CORES, C, HW)
        .astype(ml_dtypes.bfloat16)
    )
    in_maps = [
        {"top": top_all[b], "bot": bot_all[b], **weights} for b in range(N_CORES)
    ]
    res = bass_utils.run_bass_kernel_spmd(nc, in_maps, core_ids=list(range(N_CORES)))
    out = np.stack(
        [np.asarray(res.results[b]["out"], dtype=np.float32) for b in range(N_CORES)]
    )
    return out.reshape(N_CORES, C, 64, 64)
